# revision 36
# baseline (speedup 1.0000x reference)
"""Trainium2 Bass kernel for nn_BreakthroughSNN (spiking SSM LM).

Strategy (8 NeuronCores, SPMD single NEFF):
  - Data-parallel SSM: 2048 tokens (B*S) sharded 256/core. Per core, the
    4-layer x 20-step LIF recurrence keeps membrane potentials in PSUM
    (PE accumulates state/output updates; DVE/ACT/Pool share the LIF
    elementwise work so no single engine serializes the PE).
  - SSM matmuls are fp32r hi/lo pairs (host-split, device rounding exact)
    except the last layer's C/D which run single fp32r (spike flips there
    cannot cascade; measured +0.003 rel err).
  - Temporal encoding: exact fp32 sigmoid-boundary thresholds. One-hot
    built as xb[t] = (emb>=T_t) - (emb>=T_{t+1}): DVE writes the ge values
    straight into the xb slots, Pool does in-place descending diffs. The
    whole encode is interleaved into layer 0's steps (runs in its shadow).
  - Vocab-sharded projection: time-integrated spike counts (integers,
    bf16-exact) are AllGathered through a Shared-space DRAM scratchpad;
    each core computes all 2048 tokens x its 4000-vocab shard with the
    ti-chunk as the stationary matmul operand (8 PSUM banks, 4+4 double
    buffered) and the 1/T scaling + bias fused into the PSUM drain.
    Output is bf16 (host upcasts); halves the output DMA.
"""

import numpy as np
import ml_dtypes
from contextlib import ExitStack

import concourse.bass as bass
import concourse.mybir as mybir
import concourse.tile as tile
from concourse import bacc
from concourse.bass_utils import run_bass_kernel_spmd
from concourse.masks import make_identity

F32 = mybir.dt.float32
F32R = mybir.dt.float32r
BF16 = mybir.dt.bfloat16
I32 = mybir.dt.int32
OP = mybir.AluOpType
ACTF = mybir.ActivationFunctionType

NCORES = 8
TOKPC = 256          # tokens per core
BATCH, SEQ = 4, 512
DM, DS = 512, 128
T, L = 20, 4
VOC = 32000
VSH = VOC // NCORES  # 4000 vocab per core
NV = 500             # vocab cols per proj tile (8 tiles per core)
KC = DM // 128       # 4 feature chunks


def _hilo(x):
    x = np.ascontiguousarray(x, dtype=np.float32)
    u = x.view(np.uint32)
    hi = (u & np.uint32(0xFFFFF000)).view(np.float32).copy()  # keep 11 mantissa bits
    lo = (x - hi).astype(np.float32)
    return hi, lo


def _f2key(x):
    u = int(np.array(x, dtype=np.float32).view(np.uint32))
    return (u ^ 0x80000000) if u < 0x80000000 else (0xFFFFFFFF - u)


def _key2f(k):
    u = (k ^ 0x80000000) if k >= 0x80000000 else (0xFFFFFFFF - k)
    return np.array([u], dtype=np.uint32).view(np.float32)[0]


def _g32(x):
    # replicate reference fp32 pipeline: floor happens on this value
    x = np.float32(x)
    s = np.float32(1.0) / (np.float32(1.0) + np.float32(np.exp(np.float32(-x))))
    return np.float32(s * np.float32(19.0))


def _thresholds():
    """T_k = smallest fp32 x with g32(x) >= k, k=1..19 (g32 monotone)."""
    ts = []
    for k in range(1, 20):
        lo_k = _f2key(np.float32(-30.0))
        hi_k = _f2key(np.float32(30.0))
        assert _g32(_key2f(hi_k)) >= k and _g32(_key2f(lo_k)) < k
        while hi_k - lo_k > 1:
            mid = (lo_k + hi_k) // 2
            if _g32(_key2f(mid)) >= k:
                hi_k = mid
            else:
                lo_k = mid
        ts.append(float(_key2f(hi_k)))
    return ts


def _build_nc():
    nc = bacc.Bacc("TRN2", target_bir_lowering=False, debug=False, num_devices=NCORES)

    ids_d = nc.dram_tensor("ids", [2, 128, 1], I32, kind="ExternalInput")
    emb_d = nc.dram_tensor("emb", [VOC, DM], F32, kind="ExternalInput")
    pa_d = nc.dram_tensor("pa", [L, 128, 2 * 128], F32R, kind="ExternalInput")
    pb_d = nc.dram_tensor("pb", [L, 128, 2 * KC * 128], F32R, kind="ExternalInput")
    pc_d = nc.dram_tensor("pc", [L, 128, 2 * KC * 128], F32R, kind="ExternalInput")
    pd_d = nc.dram_tensor("pd", [L, 128, 2 * KC * 128], F32R, kind="ExternalInput")
    NVG = 16              # vocab groups of 2000 cols, streamed from DRAM
    wpt_d = nc.dram_tensor("wpt", [NVG, 128, KC * 2000], BF16,
                           kind="ExternalInput")
    bias_d = nc.dram_tensor("bias", [1, VOC], BF16, kind="ExternalInput")
    out_d = nc.dram_tensor("out", [TOKPC, VOC], BF16, kind="ExternalOutput")

    THR = _thresholds()

    with tile.TileContext(nc) as tc, ExitStack() as ctx:
        const = ctx.enter_context(tc.tile_pool(name="const", bufs=1))
        ident = const.tile([128, 128], F32)
        make_identity(nc, ident[:])
        ident_r = const.tile([128, 128], F32R)
        nc.vector.tensor_copy(ident_r[:], ident[:])
        neg2 = const.tile([128, 1], F32)
        nc.vector.memset(neg2[:], -2.0)
        half_c = const.tile([128, 1], F32)
        nc.vector.memset(half_c[:], 0.5)
        one_c = const.tile([128, 1], F32)
        nc.vector.memset(one_c[:], 1.0)

        xb_pool = ctx.enter_context(tc.tile_pool(name="xb", bufs=1))
        # xb layout: [128, T * 1024]; step t block = 4 feature chunks x 256 toks
        xb = xb_pool.tile([128, T * KC * 256], F32R)

        tip = ctx.enter_context(tc.tile_pool(name="ti", bufs=1))
        tibf = tip.tile([128, KC * 256], BF16, tag="tibf")

        # all SSM-scoped SBUF lives in sctx, freed before the projection
        sctx = ctx.enter_context(ExitStack())

        # ids first on the sync DMA queue: everything pre-SSM hangs off it
        enc = sctx.enter_context(tc.tile_pool(name="enc", bufs=2))
        ids_s = enc.tile([128, 2], I32, tag="ids")
        for g in range(2):
            nc.sync.dma_start(ids_s[:, g:g + 1], ids_d[g, :, :])

        par = sctx.enter_context(tc.tile_pool(name="par", bufs=4))

        def load_layer(layer):
            pa_t = par.tile([128, 2 * 128], F32R, tag="pa")
            pb_t = par.tile([128, 2 * KC * 128], F32R, tag="pb")
            pc_t = par.tile([128, 2 * KC * 128], F32R, tag="pc")
            pd_t = par.tile([128, 2 * KC * 128], F32R, tag="pd")
            nc.sync.dma_start(pa_t[:], pa_d[layer, :, :])
            nc.sync.dma_start(pb_t[:], pb_d[layer, :, :])
            nc.sync.dma_start(pc_t[:], pc_d[layer, :, :])
            nc.sync.dma_start(pd_t[:], pd_d[layer, :, :])
            return pa_t, pb_t, pc_t, pd_t

        params = {0: load_layer(0), 1: load_layer(1)}

        # ---------------- embedding gather + transpose ----------------------
        emb4 = sctx.enter_context(tc.tile_pool(name="emb4", bufs=1))
        EMB = emb4.tile([128, KC * 256], F32)  # [feat-in-chunk, k*256 + tok]
        with tc.tile_pool(name="encp", bufs=2, space="PSUM") as encps:
            for g in range(2):
                eg = enc.tile([128, DM], F32, tag="eg")
                nc.gpsimd.indirect_dma_start(
                    out=eg[:], out_offset=None,
                    in_=emb_d[:, :],
                    in_offset=bass.IndirectOffsetOnAxis(ap=ids_s[:, g:g + 1], axis=0),
                )
                for k in range(KC):
                    pt = encps.tile([128, 128], F32, tag="pt")
                    nc.tensor.transpose(pt[:], eg[:, k * 128:(k + 1) * 128], ident[:])
                    nc.scalar.copy(EMB[:, k * 256 + g * 128:k * 256 + g * 128 + 128],
                                   pt[:])

        # ---------------- temporal one-hot encode ---------------------------
        # ge_s = (EMB >= THR[s-1]) written into xb[s]; then (Pool, ascending)
        # xb[s] -= xb[s+1]_raw. xb[19] stays raw (ge_20 == 0). xb[0] = 1-ge_1.
        def xslot(s):
            return xb[:, s * KC * 256:(s + 1) * KC * 256]

        def emit_ge(s):  # DVE: raw ge_s into xb[s]
            nc.vector.tensor_scalar(xslot(s), EMB[:], float(THR[s - 1]), None,
                                    OP.is_ge)

        def emit_sub(s):  # Pool: finalize xb[s] (s=1..18)
            nc.gpsimd.tensor_tensor(xslot(s), xslot(s), xslot(s + 1), OP.subtract)

        # head: xb[0], xb[1], xb[2] final before layer-0 steps 0..2 need them
        emit_ge(1)
        emit_ge(2)
        # xb[0] = 1 - ge_1 (reads xb[1] raw; runs before sub(1) on Pool)
        nc.gpsimd.tensor_scalar(xslot(0), xslot(1), -1.0, 1.0, OP.mult, OP.add)
        emit_ge(3)
        emit_sub(1)
        emit_sub(2)

        # ---------------- SSM layers: 2-layer pipelined interleave ----------
        # Layers run in pairs (0,1) then (2,3); the second layer of a pair
        # trails the first by 2 steps. Each layer's LIF chains hide behind
        # the other layer's independent matmuls, keeping the PE saturated.
        params[2] = load_layer(2)
        params[3] = load_layer(3)
        with tc.tile_pool(name="ssmp", bufs=1, space="PSUM") as ssmps, \
             sctx, tc.tile_pool(name="lif", bufs=2) as lif:
            v1f = {s: ssmps.tile([128, 512], F32, tag=f"v1{s}", name=f"v1{s}")
                   for s in "ab"}
            v2f = {s: [ssmps.tile([128, 2 * TOKPC], F32, tag=f"v2{s}{j}",
                                  name=f"v2{s}{j}") for j in range(2)]
                   for s in "ab"}
            tips = ssmps.tile([128, KC * TOKPC], F32, tag="tips")

            def psl(tile_, h, k):
                return tile_[:, (h * KC + k) * 128:(h * KC + k) * 128 + 128]

            def make_state(side, layer):
                pa_t, pb_t, pc_t, pd_t = params[layer]
                return {
                    "side": side, "layer": layer,
                    "v1": v1f[side][:, 0:TOKPC], "v2": v2f[side],
                    "ah": pa_t[:, 0:128], "al": pa_t[:, 128:256],
                    "bh": [psl(pb_t, 0, k) for k in range(KC)],
                    "bl": [psl(pb_t, 1, k) for k in range(KC)],
                    "ch": [psl(pc_t, 0, k) for k in range(KC)],
                    "cl": [psl(pc_t, 1, k) for k in range(KC)],
                    "ddh": [psl(pd_t, 0, k) for k in range(KC)],
                    "ddl": [psl(pd_t, 1, k) for k in range(KC)],
                    "H": {},
                }

            def emit_mm1_lif1(S, t):
                xs = [xb[:, (t * KC + k) * 256:(t * KC + k) * 256 + 256]
                      for k in range(KC)]
                mm1 = []
                for k in range(KC):
                    mm1 += [(S["bh"][k], xs[k]), (S["bl"][k], xs[k])]
                if t > 0:
                    mm1 += [(S["ah"], S["H"][t - 1][:]),
                            (S["al"], S["H"][t - 1][:])]
                for i, (lhsT, rhs) in enumerate(mm1):
                    nc.tensor.matmul(S["v1"], lhsT, rhs,
                                     start=(t == 0 and i == 0),
                                     stop=(i == len(mm1) - 1),
                                     skip_group_check=True)
                H = lif.tile([128, TOKPC], F32R, tag=f"H{S['side']}",
                             name=f"H{S['side']}", bufs=3)
                nc.vector.tensor_scalar(H[:], S["v1"], 2.0, None, OP.is_ge)
                if t < T - 1:
                    m1 = lif.tile([128, TOKPC], F32, tag=f"m1{S['side']}",
                                  name=f"m1{S['side']}")
                    nc.scalar.activation(m1[:], H[:].bitcast(F32),
                                         ACTF.Copy, bias=0.5, scale=-0.5)
                    nc.vector.tensor_tensor(S["v1"], S["v1"], m1[:], OP.mult)
                S["H"][t] = H
                S["H"].pop(t - 2, None)

            def emit_out(S, t):
                layer, side = S["layer"], S["side"]
                v2pr = S["v2"]
                H_t = S["H"][t]
                last = layer == L - 1
                xs = [xb[:, (t * KC + k) * 256:(t * KC + k) * 256 + 256]
                      for k in range(KC)]
                for k in range(KC):
                    vsl = v2pr[k // 2][:, (k % 2) * TOKPC:(k % 2 + 1) * TOKPC]
                    if last:
                        mm2 = [(S["ch"][k], H_t[:]), (S["ddh"][k], xs[k])]
                    else:
                        mm2 = [(S["ch"][k], H_t[:]), (S["cl"][k], H_t[:]),
                               (S["ddh"][k], xs[k]), (S["ddl"][k], xs[k])]
                    for i, (lhsT, rhs) in enumerate(mm2):
                        nc.tensor.matmul(vsl, lhsT, rhs,
                                         start=(t == 0 and i == 0 and k % 2 == 0),
                                         stop=(i == len(mm2) - 1),
                                         skip_group_check=True)
                # LIF2: ACT sign -> Pool mask -> ACT/Pool spike-out -> DVE decay
                notlast_t = t < T - 1
                sgs, m2s = [], []
                for j in range(2):
                    sg = lif.tile([128, 2 * TOKPC], F32, tag=f"sg{side}{j}",
                                  name=f"sg{side}{j}")
                    nc.scalar.activation(sg[:], v2pr[j][:], ACTF.Sign,
                                         bias=neg2[:], scale=1.0)
                    sgs.append(sg)
                    if notlast_t:
                        m2 = lif.tile([128, 2 * TOKPC], F32, tag=f"m2{side}{j}",
                                      name=f"m2{side}{j}")
                        nc.gpsimd.tensor_scalar(m2[:], sg[:], -0.25, 0.25,
                                                OP.mult, OP.add)
                        m2s.append(m2)
                for j in range(2):
                    xsl = xb[:, t * 1024 + j * 512:t * 1024 + j * 512 + 512]
                    # spike-out engine per layer: keep Pool clear of layer-0
                    # slots (encode diffs) and ACT clear in pair 2
                    if layer == 0 or (layer == 1 and j == 0):
                        nc.scalar.activation(xsl, sgs[j][:], ACTF.Copy,
                                             bias=0.5, scale=0.5)
                    else:
                        nc.gpsimd.tensor_scalar(xsl, sgs[j][:], 0.5, 0.5,
                                                OP.mult, OP.add)
                if notlast_t:
                    for j in range(2):
                        nc.vector.tensor_tensor(v2pr[j][:], v2pr[j][:],
                                                m2s[j][:], OP.mult)

            def emit_tips(t):
                # time-integration on the PE: tips += I @ X[t] (X = layer-3 out)
                for k in range(KC):
                    nc.tensor.matmul(
                        tips[:, k * TOKPC:(k + 1) * TOKPC],
                        ident_r[:], xb[:, (t * KC + k) * 256:(t * KC + k) * 256 + 256],
                        start=(t == 0 and k % 2 == 0),
                        stop=(t == T - 1),
                        skip_group_check=True)

            for la, lb in [(0, 1), (2, 3)]:
                SA = make_state("a", la)
                SB = make_state("b", lb)
                for slot in range(T + 4):
                    ta, tb, tq = slot, slot - 2, slot - 4
                    if ta < T:
                        if la == 0:
                            if ta + 4 <= 19:
                                emit_ge(ta + 4)
                            if ta + 3 <= 18:
                                emit_sub(ta + 3)
                        emit_mm1_lif1(SA, ta)
                    if 1 <= ta <= T:
                        emit_out(SA, ta - 1)
                    if 0 <= tb < T:
                        emit_mm1_lif1(SB, tb)
                    if 1 <= tb <= T:
                        emit_out(SB, tb - 1)
                    if lb == L - 1 and 0 <= tq < T:
                        emit_tips(tq)

            # spike-count integral -> bf16 (integers 0..20, bf16-exact;
            # the 1/T scaling is fused into the projection drain)
            nc.vector.tensor_copy(tibf[:], tips[:])

        # ---------------- token-local projection (no collective) ------------
        # Each core projects only its own 256 tokens against the FULL vocab,
        # streaming Wp from DRAM in 2MB groups (hidden under the matmuls).
        with tc.tile_pool(name="prjp", bufs=2, space="PSUM") as prjps, \
             tc.tile_pool(name="wstr", bufs=5) as wstr, \
             tc.tile_pool(name="bstr", bufs=3) as bstr, \
             tc.tile_pool(name="osb", bufs=3) as osbp:

            def load_group(g):
                wt = wstr.tile([128, KC * 2000], BF16, tag="wt")
                # alternate DMA queues so the 2MB weight stream keeps pace
                eng = nc.sync if g % 2 == 0 else nc.scalar
                eng.dma_start(wt[:], wpt_d[g, :, :])
                bt = bstr.tile([128, 2000], BF16, tag="bt")
                bap = bias_d[0:1, g * 2000:(g + 1) * 2000]
                bsrc = bass.AP(tensor=bap.tensor, offset=bap.offset,
                               ap=[[0, 128], [1, 2000]])
                nc.gpsimd.dma_start(bt[:], bsrc)
                return wt, bt

            wgroups = {g: load_group(g) for g in range(4)}
            for g in range(NVG):
                if g + 4 < NVG:
                    wgroups[g + 4] = load_group(g + 4)
                wt, bt = wgroups.pop(g)
                for m in range(2):
                    po = [prjps.tile([128, NV], F32, tag=f"po{i}", name=f"po{i}")
                          for i in range(4)]
                    for k in range(KC):
                        lh = tibf[:, k * 256 + m * 128:k * 256 + m * 128 + 128]
                        for i in range(4):
                            nc.tensor.matmul(
                                po[i][:], lh,
                                wt[:, k * 2000 + i * NV:k * 2000 + i * NV + NV],
                                start=(k == 0), stop=(k == KC - 1),
                                skip_group_check=True)
                    osb = osbp.tile([128, 4 * NV], BF16, tag="osb")
                    for i in range(4):
                        nc.vector.scalar_tensor_tensor(
                            osb[:, i * NV:(i + 1) * NV], po[i][:], 1.0 / T,
                            bt[:, i * NV:(i + 1) * NV], OP.mult, OP.add)
                    nc.gpsimd.dma_start(
                        out_d[m * 128:(m + 1) * 128,
                              g * 2000:(g + 1) * 2000], osb[:])

    nc.compile()
    return nc


_NC_CACHE = {}
_last_in_maps = None


def _get_nc():
    if "nc" not in _NC_CACHE:
        _NC_CACHE["nc"] = _build_nc()
    return _NC_CACHE["nc"]


def kernel(input_ids, emb_table, A, B, C, D, Wp, bp):
    input_ids = np.asarray(input_ids)
    emb_table = np.ascontiguousarray(np.asarray(emb_table), dtype=np.float32)
    A = np.asarray(A, dtype=np.float32)
    B = np.asarray(B, dtype=np.float32)
    C = np.asarray(C, dtype=np.float32)
    D = np.asarray(D, dtype=np.float32)
    Wp = np.asarray(Wp, dtype=np.float32)
    bp = np.asarray(bp, dtype=np.float32)

    ids_flat = input_ids.reshape(-1).astype(np.int32)          # (2048,)

    at = np.ascontiguousarray(A.transpose(0, 2, 1))            # (L,128,128)
    at_hi, at_lo = _hilo(at)
    pa = np.ascontiguousarray(
        np.stack([at_hi, at_lo], axis=2).reshape(L, 128, 2 * 128))

    bt = np.ascontiguousarray(
        B.transpose(2, 0, 1).reshape(KC, 128, L, DS).transpose(2, 1, 0, 3))
    # bt[l,p,k,m] = B[l, m, k*128+p]
    bt_hi, bt_lo = _hilo(bt)
    pb = np.ascontiguousarray(
        np.stack([bt_hi, bt_lo], axis=2).reshape(L, 128, 2 * KC * 128))

    ct = np.ascontiguousarray(C.transpose(0, 2, 1).reshape(L, 128, KC, 128))
    # ct[l,p,mc,m] = C[l, mc*128+m, p]
    ct_hi, ct_lo = _hilo(ct)
    # last layer: single fp32r -> feed the full value (device rounds)
    ct_hi[L - 1] = ct[L - 1]
    pc = np.ascontiguousarray(
        np.stack([ct_hi, ct_lo], axis=2).reshape(L, 128, 2 * KC * 128))

    dh, dl = _hilo(D)                                          # (L, 512)
    dh = dh.reshape(L, KC, 128).copy()
    dl = dl.reshape(L, KC, 128)
    dh[L - 1] = D.reshape(L, KC, 128)[L - 1]
    pd = np.zeros((L, 128, 2, KC, 128), np.float32)
    i = np.arange(128)
    pd[:, i, 0, :, i] = dh.transpose(2, 0, 1)                  # (128, L, KC)
    pd[:, i, 1, :, i] = dl.transpose(2, 0, 1)
    pd = np.ascontiguousarray(pd.reshape(L, 128, 2 * KC * 128))

    bp_bf = np.ascontiguousarray(bp.astype(ml_dtypes.bfloat16).reshape(1, VOC))

    # wpt[g, p, k*2000 + v] = Wp[g*2000 + v, k*128 + p]
    NVG = 16
    wpt_h = Wp.reshape(NVG, 2000, KC, 128).transpose(0, 3, 2, 1)
    wpt_h = np.ascontiguousarray(wpt_h.reshape(NVG, 128, KC * 2000)) \
        .astype(ml_dtypes.bfloat16)

    nc = _get_nc()
    in_maps = []
    for c in range(NCORES):
        ids_c = ids_flat[c * TOKPC:(c + 1) * TOKPC].reshape(2, 128, 1)
        in_maps.append({
            "ids": np.ascontiguousarray(ids_c),
            "emb": emb_table,
            "pa": pa, "pb": pb, "pc": pc, "pd": pd,
            "wpt": wpt_h,
            "bias": bp_bf,
        })

    global _last_in_maps
    _last_in_maps = in_maps
    res = run_bass_kernel_spmd(nc, in_maps, core_ids=list(range(NCORES)))
    outs = [res.results[c]["out"].astype(np.float32) for c in range(NCORES)]
    full = np.concatenate(outs, axis=0)                        # (2048, 32000)
    return full.reshape(BATCH, SEQ, VOC).astype(np.float32)
